# revision 45
# baseline (speedup 1.0000x reference)
"""Distributed Trainium2 attention kernel (8 NeuronCores, head-sharded TP).

Reference computation (per problem spec, hardcoded shapes):
  B=2, S=2048, HID=2048, H=32 q-heads, HKV=8 kv-heads, D=64, GQA ratio 4.
  q/k/v projections -> RoPE(q,k) -> causal softmax attention -> out proj wo.

Sharding: tensor-parallel over heads. Core i owns q-heads 4i..4i+3 and
kv-head i (GQA groups stay aligned). Everything on-device is computed in a
transposed layout ([feature, seq]) so no transposes are needed anywhere
except a PE-transpose (identity matmul) for V.

  - host passes hidden in [HID-chunk, group, 128, 1024] contiguous blocks,
    wqT [HID, 256], wkvT [HID, 128], woT [H*D, HID] (all bf16), rope tables
    cosT/sinT in [d, s]-major layout with rotate-half signs folded in, and a
    single [128,128] triangular causal band mask.
  - qT = wqT.T @ hiddenT, kT/vT packed: [wk|wv].T @ hiddenT (PSUM accum)
  - RoPE in [d, s] layout via partition-swap DMA + 3 DVE ops.
  - attention processes HEAD PAIRS: two scoresT matmuls (contraction K=64)
    run concurrently in the PE array at row offsets 0/64, into one
    [128, 1024] PSUM pair; one ACT exp covers both; causal masking is a
    [128,128] band add pre-exp plus bf16 zero-memsets post-exp.
  - PV: stationary [v | ones] [128, 65] -> out2T [65, 512] PSUM accum;
    row 64 is the softmax denominator for free.
  - divide: DVE reciprocal -> DMA to partition 0 -> gpsimd
    partition_broadcast -> DVE mul (bf16)
  - TWO AllToAlls (head pair 0/1) convert head-sharding -> seq-sharding;
    the first overlaps the second half of attention.
  - wo matmul accumulates interleaved chunks (A2A#1 chunks first) so it can
    start before A2A#2 completes. Host concatenates the 8 [512, 2048] f32
    output shards.
"""

import os
import sys

import numpy as np
import ml_dtypes

sys.path.insert(0, "/opt/trn_rl_repo")

import concourse.bass as bass  # noqa: E402
import concourse.mybir as mybir  # noqa: E402
import concourse.tile as tile  # noqa: E402
from concourse import bacc  # noqa: E402
from concourse.bass_utils import run_bass_kernel_spmd  # noqa: E402

F32 = mybir.dt.float32
BF16 = mybir.dt.bfloat16

H, HKV, D = 32, 8, 64
HID = 2048
B, S = 2, 2048
BS = B * S            # 4096 flattened (b, s)
NCORES = 8
NHQ = H // NCORES     # 4 local q heads
MQ = NHQ * D          # 256 local q rows
SCHUNK = BS // NCORES  # 512 output rows per core
NEG = -30000.0

LAST_EXEC_NS = None


def _build():
    nc = bacc.Bacc("TRN2", target_bir_lowering=False, debug=False,
                   num_devices=NCORES)

    hidden4 = nc.dram_tensor("hidden4", [8, 16, 128, 512], BF16,
                             kind="ExternalInput")
    wqT = nc.dram_tensor("wqT", [HID, MQ], BF16, kind="ExternalInput")
    wkvT = nc.dram_tensor("wkvT", [HID, 2 * D], BF16, kind="ExternalInput")
    woT = nc.dram_tensor("woT", [H * D, HID], BF16, kind="ExternalInput")
    cosT = nc.dram_tensor("cosT", [128, BS], BF16, kind="ExternalInput")
    sinT = nc.dram_tensor("sinT", [128, BS], BF16, kind="ExternalInput")
    maskb = nc.dram_tensor("maskb", [128, 128], BF16, kind="ExternalInput")
    ident = nc.dram_tensor("ident", [128, D], BF16, kind="ExternalInput")
    out = nc.dram_tensor("out", [SCHUNK, HID], F32, kind="ExternalOutput")

    with tile.TileContext(nc) as tc:
        with (
            tc.tile_pool(name="persist", bufs=1) as persist,
            tc.tile_pool(name="qkv", bufs=1) as qkv,
            tc.tile_pool(name="wop", bufs=1) as wop,
            tc.tile_pool(name="dram", bufs=1, space="DRAM") as dram,
        ):
            # ---- persistent SBUF loads -------------------------------------
            # matmul weights first on the sync queue (they gate the first
            # projection); rope tables etc. on the scalar queue.
            wq_sb = persist.tile([128, 16, MQ], BF16)
            nc.sync.dma_start(wq_sb, wqT.rearrange("(c p) m -> p c m", p=128))
            wkv_sb = persist.tile([128, 16, 2 * D], BF16)
            nc.scalar.dma_start(wkv_sb,
                                wkvT.rearrange("(c p) m -> p c m", p=128))
            cos_sb = persist.tile([128, BS], BF16)
            nc.sync.dma_start(cos_sb, cosT[:, :])
            sin_sb = persist.tile([128, BS], BF16)
            nc.sync.dma_start(sin_sb, sinT[:, :])
            mask_sb = persist.tile([128, 128], BF16)
            nc.sync.dma_start(mask_sb, maskb[:, :])
            id_sb = persist.tile([128, D], BF16)
            nc.sync.dma_start(id_sb, ident[:, :])

            # wo weights go through the gpsimd SWDGE queue in chunks so their
            # long transfer never sits on a HWDGE queue whose cumulative
            # semaphore gates small latency-critical DMAs.
            wo_sb = wop.tile([128, 16, HID], BF16)

            # ---- qkv outputs ----------------------------------------------
            qrope = qkv.tile([128, 2, BS], BF16)     # [64*(h%2), h//2, b*S+s]
            krope = qkv.tile([128, BS], BF16)        # duplicated both halves
            vT_sb = qkv.tile([128, BS], BF16)        # rows 64:128 used
            # per k-tile block: col 0 = ones (denominator row lands on PSUM
            # partition 0 so partition_broadcast works directly), cols 1:65 = v
            v_sb = qkv.tile([128, B, 16 * (D + 1)], BF16)
            for b in range(B):
                for kt in range(16):
                    nc.vector.memset(v_sb[:, b, kt * 65 + 64: kt * 65 + 65],
                                     1.0)

            cc_in_a = dram.tile([NCORES, 128, 512], BF16)  # heads 0,1
            cc_in_b = dram.tile([NCORES, 128, 512], BF16)  # heads 2,3
            cc_out_a = dram.tile([NCORES * 128, 512], BF16)
            cc_out_b = dram.tile([NCORES * 128, 512], BF16)

            # ================= phase 1: projections + rope ==================
            with (
                tc.tile_pool(name="hstream", bufs=2) as hstream,
                tc.tile_pool(name="ropetmp", bufs=2) as ropetmp,
                tc.tile_pool(name="psA", bufs=2, space="PSUM") as psA,
                tc.tile_pool(name="psT", bufs=2, space="PSUM") as psT,
            ):
                h_dmas = []
                for g in range(8):  # 512-col groups of (b, s)
                    cols = bass.ds(g * 512, 512)
                    ps_q0 = psA.tile([128, 512], F32, tag="q0")
                    ps_q1 = psA.tile([128, 512], F32, tag="q1")
                    ps_kv = psA.tile([128, 512], F32, tag="kv")
                    h_sb = hstream.tile([128, 16, 512], BF16, tag="h")
                    if g == 0:
                        # split the first load so c=0..7 matmuls start sooner
                        nc.scalar.dma_start(
                            h_sb[:, 0:8, :], hidden4[g, 0:8, :, :]
                            .rearrange("c p n -> p c n"))
                        h_dma = nc.scalar.dma_start(
                            h_sb[:, 8:16, :], hidden4[g, 8:16, :, :]
                            .rearrange("c p n -> p c n"))
                    else:
                        h_dma = nc.scalar.dma_start(
                            h_sb, hidden4[g, :, :, :]
                            .rearrange("c p n -> p c n"))
                    h_dmas.append(h_dma)
                    for c in range(16):
                        nc.tensor.matmul(ps_q0, wq_sb[:, c, 0:128],
                                         h_sb[:, c, :],
                                         start=(c == 0), stop=(c == 15))
                        nc.tensor.matmul(ps_q1, wq_sb[:, c, 128:256],
                                         h_sb[:, c, :],
                                         start=(c == 0), stop=(c == 15))
                        nc.tensor.matmul(ps_kv, wkv_sb[:, c, :], h_sb[:, c, :],
                                         start=(c == 0), stop=(c == 15))
                    # rope for q0/q1/kv batched: x = [q0 | q1 | kv] bf16,
                    # one stepped-AP DMA pair does all partition swaps.
                    x = ropetmp.tile([128, 3, 512], BF16, tag="x")
                    nc.scalar.copy(x[:, 0, :], ps_q0)
                    nc.scalar.copy(x[:, 1, :], ps_q1)
                    nc.scalar.copy(x[0:64, 2, :], ps_kv[0:64, :])
                    swap = ropetmp.tile([128, 3, 512], BF16, tag="swap")
                    for blk in range(2):
                        p0 = blk * 64
                        nc.scalar.dma_start(swap[p0: p0 + 32, :, :],
                                            x[p0 + 32: p0 + 64, :, :])
                        nc.scalar.dma_start(swap[p0 + 32: p0 + 64, :, :],
                                            x[p0: p0 + 32, :, :])
                    tmp = ropetmp.tile([128, 3, 512], BF16, tag="tmp")
                    cosp = ropetmp.tile([128, 3, 512], BF16, tag="cosp")
                    for t, rows in ((0, 128), (1, 128), (2, 64)):
                        nc.vector.tensor_mul(tmp[:rows, t, :],
                                             swap[:rows, t, :],
                                             sin_sb[:rows, cols])
                        nc.vector.tensor_mul(cosp[:rows, t, :],
                                             x[:rows, t, :],
                                             cos_sb[:rows, cols])
                    nc.vector.tensor_add(qrope[:, 0, cols], tmp[:, 0, :],
                                         cosp[:, 0, :])
                    nc.vector.tensor_add(qrope[:, 1, cols], tmp[:, 1, :],
                                         cosp[:, 1, :])
                    nc.vector.tensor_add(krope[0:64, cols], tmp[0:64, 2, :],
                                         cosp[0:64, 2, :])
                    nc.scalar.copy(vT_sb[64:128, cols], ps_kv[64:128, :])
                    b = g // 4
                    for j in range(4):
                        st = g * 512 + j * 128
                        kt = (st - b * S) // 128
                        tp = psT.tile([128, D], BF16, tag="tp")
                        nc.tensor.transpose(tp,
                                            vT_sb[64:128, bass.ds(st, 128)],
                                            id_sb[64:128, :])
                        nc.vector.tensor_copy(
                            v_sb[:, b, kt * 65: kt * 65 + 64], tp)
                # duplicate k into partitions 64:128 (one DMA per batch)
                for b in range(B):
                    bc = bass.ds(b * S, S)
                    nc.sync.dma_start(krope[64:128, bc], krope[0:64, bc])

            # woT load (SWDGE, chunked); held behind the last hidden load
            # so its transfers don't starve the phase-1 stream.
            from concourse.tile import add_dep_helper
            for ch in range(4):
                wo_dma = nc.gpsimd.dma_start(
                    wo_sb[:, bass.ts(ch, 4), :],
                    woT.rearrange("(c p) n -> p c n", p=128)[:,
                                                            bass.ts(ch, 4), :])
                add_dep_helper(wo_dma.ins, h_dmas[4 + ch].ins, sync=True,
                               reason="woT chunk interleaves hidden tail")



            # ================= phase 2: attention ===========================
            # head pairs: hp=0 -> heads 0,1 (qrope m=0), hp=1 -> heads 2,3
            with (
                tc.tile_pool(name="attnp", bufs=4) as attnp,
                tc.tile_pool(name="divp", bufs=3) as divp,
                tc.tile_pool(name="psS", bufs=3, space="PSUM") as psS,
                tc.tile_pool(name="psO", bufs=1, space="PSUM") as psO,
            ):
                for hp in range(2):
                    cc_tile = cc_in_a if hp == 0 else cc_in_b
                    for b in range(B):
                        for qc in range(4):
                            qcols = bass.ds(b * S + qc * 512, 512)
                            nkt = 4 * qc + 4
                            ps_oa = psO.tile([65, 512], F32, tag="poa")
                            ps_ob = psO.tile([65, 512], F32, tag="pob")
                            for kt in range(nkt):
                                kcols = bass.ds(b * S + kt * 128, 128)
                                ps_s = psS.tile([128, 1024], F32, tag="pss")
                                nc.tensor.matmul(ps_s[:, 0:512],
                                                 krope[0:64, kcols],
                                                 qrope[0:64, hp, qcols],
                                                 start=True, stop=True)
                                nc.tensor.matmul(ps_s[:, 512:1024],
                                                 krope[64:128, kcols],
                                                 qrope[64:128, hp, qcols],
                                                 start=True, stop=True)
                                j = kt - 4 * qc
                                attn = attnp.tile([128, 1024], BF16,
                                                  tag="attn")
                                nc.scalar.activation(
                                    attn, ps_s,
                                    mybir.ActivationFunctionType.Exp,
                                    scale=0.125)
                                if j >= 0:
                                    # causal band: multiply by 0/1 mask
                                    # post-exp (keeps DVE off the
                                    # scores->exp critical path)
                                    for half in range(2):
                                        o = half * 512 + j * 128
                                        nc.vector.tensor_mul(
                                            attn[:, o:o + 128],
                                            attn[:, o:o + 128], mask_sb)
                                if j >= 1:
                                    for half in range(2):
                                        o = half * 512
                                        nc.vector.memset(
                                            attn[:, o:o + j * 128], 0.0)
                                vs = v_sb[:, b, kt * 65: kt * 65 + 65]
                                nc.tensor.matmul(ps_oa, vs, attn[:, 0:512],
                                                 start=(kt == 0),
                                                 stop=(kt == nkt - 1),
                                                 skip_group_check=True)
                                nc.tensor.matmul(ps_ob, vs, attn[:, 512:1024],
                                                 start=(kt == 0),
                                                 stop=(kt == nkt - 1),
                                                 skip_group_check=True)
                            for half, ps_o in ((0, ps_oa), (1, ps_ob)):
                                # copy out of PSUM right away so the bank is
                                # free for the next q-chunk's accumulation
                                osb = divp.tile([65, 512], F32, tag="osb")
                                nc.vector.tensor_copy(osb, ps_o)
                                ps_o = osb
                                recip = divp.tile([65, 512], F32, tag="recip")
                                nc.vector.reciprocal(recip[64:65, :],
                                                     ps_o[64:65, :])
                                # broadcast across partitions via a stride-0
                                # DRAM read (keeps the Pool engine free for
                                # the collectives)
                                den_d = dram.tile([1, 512], F32, tag="den",
                                                  bufs=4, name="den_d")
                                nc.scalar.dma_start(den_d, recip[64:65, :])
                                rbc = divp.tile([64, 512], F32, tag="rbc")
                                nc.scalar.dma_start(
                                    rbc, den_d[0:1, :].broadcast_to((64, 512)))
                                ao = divp.tile([64, 512], BF16, tag="ao")
                                nc.vector.tensor_mul(ao, ps_o[0:64, :], rbc)
                                nc.sync.dma_start(
                                    cc_tile[b * 4 + qc,
                                            bass.ts(half, 64), :], ao)
                    # fire the A2A for this head pair as soon as its last
                    # bounce write lands; hp=0's overlaps hp=1's attention.
                    nc.gpsimd.collective_compute(
                        "AllToAll", mybir.AluOpType.bypass,
                        replica_groups=[list(range(NCORES))],
                        ins=[(cc_in_a if hp == 0 else cc_in_b).opt()],
                        outs=[(cc_out_a if hp == 0 else cc_out_b).opt()])
                    # aot loads via SWDGE (off the HWDGE queues so their
                    # A2A gating can't poison cumulative queue semaphores);
                    # emitted here so the trigger isn't stuck behind the
                    # next A2A's input wait on the Pool SEQ.
                    if hp == 0:
                        aot_a = wop.tile([128, 8, 512], BF16)
                        nc.gpsimd.dma_start(
                            aot_a,
                            cc_out_a.rearrange("(c p) n -> p c n", p=128))
                    else:
                        aot_b = wop.tile([128, 8, 512], BF16)
                        nc.gpsimd.dma_start(
                            aot_b,
                            cc_out_b.rearrange("(c p) n -> p c n", p=128))


            # ================= phase 3: wo ==================================
            with (
                tc.tile_pool(name="psW", bufs=2, space="PSUM") as psW,
                tc.tile_pool(name="outp", bufs=2) as outp,
            ):
                # two st-waves of 2; within a wave all A2A#1 (even) chunks
                # first so the PE isn't head-of-line blocked on A2A#2.
                for wave in range(2):
                    pws = [psW.tile([128, HID], F32, tag=f"psw{w}", bufs=1,
                                    name=f"psw_{wave}_{w}")
                           for w in range(2)]
                    for phase, aot in ((0, aot_a), (1, aot_b)):
                        for w in range(2):
                            st = wave * 2 + w
                            for i in range(8):
                                c = 2 * i + phase
                                for nh in range(4):
                                    ns = bass.ts(nh, 512)
                                    nc.tensor.matmul(
                                        pws[w][:, ns],
                                        aot[:, i, bass.ts(st, 128)],
                                        wo_sb[:, c, ns],
                                        start=(phase == 0 and i == 0),
                                        stop=(phase == 1 and i == 7))
                    for w in range(2):
                        st = wave * 2 + w
                        osb = outp.tile([128, HID], F32, tag="osb")
                        nc.vector.tensor_copy(osb, pws[w])
                        nc.sync.dma_start(out[bass.ts(st, 128), :], osb)

    nc.compile()
    return nc


_NC_CACHE = {}


def _get_nc():
    if "nc" not in _NC_CACHE:
        _NC_CACHE["nc"] = _build()
    return _NC_CACHE["nc"]


def _prep_inputs(hidden_states, cos, sin, wq, wk, wv, wo):
    bf = ml_dtypes.bfloat16
    hiddenT = np.ascontiguousarray(
        hidden_states.reshape(BS, HID).T).astype(bf)       # [HID, BS]
    hidden4 = np.ascontiguousarray(
        hiddenT.reshape(16, 128, 8, 512).transpose(2, 0, 1, 3))
    woT = np.ascontiguousarray(np.asarray(wo).T).astype(bf)

    cos2 = np.asarray(cos)[:, 0, :]          # [S, D]
    sin2 = np.asarray(sin)[:, 0, :]
    cosTb = cos2.T                            # [D, S]
    sinTb = sin2.T
    sin_signed = np.concatenate([-sinTb[:32], sinTb[32:]], axis=0)
    cos_full = np.tile(cosTb, (2, B)).astype(bf)       # [128, B*S]
    sin_full = np.tile(sin_signed, (2, B)).astype(bf)  # [128, B*S]

    # triangular causal band mask for the diagonal [128k x 128q] block
    kk = np.arange(128)[:, None]
    qq = np.arange(128)[None, :]
    maskb = np.where(kk > qq, 0.0, 1.0).astype(np.float32).astype(bf)

    ident_np = np.zeros((128, D), np.float32)
    ident_np[64:128, :] = np.eye(D)
    ident_np = ident_np.astype(bf)

    wq = np.asarray(wq)
    wk = np.asarray(wk)
    wv = np.asarray(wv)
    in_maps = []
    for i in range(NCORES):
        wq_i = wq[i * MQ:(i + 1) * MQ, :]                      # [256, HID]
        wkv_i = np.concatenate([wk[i * D:(i + 1) * D, :],
                                wv[i * D:(i + 1) * D, :]], axis=0)
        in_maps.append({
            "hidden4": hidden4,
            "wqT": np.ascontiguousarray(wq_i.T).astype(bf),
            "wkvT": np.ascontiguousarray(wkv_i.T).astype(bf),
            "woT": woT,
            "cosT": cos_full,
            "sinT": sin_full,
            "maskb": maskb,
            "ident": ident_np,
        })
    return in_maps


def kernel(hidden_states, cos, sin, wq, wk, wv, wo):
    global LAST_EXEC_NS
    in_maps = _prep_inputs(np.asarray(hidden_states, np.float32),
                           cos, sin, wq, wk, wv, wo)
    nc = _get_nc()
    res = run_bass_kernel_spmd(nc, in_maps, core_ids=list(range(NCORES)),
                               trace=bool(int(os.environ.get("BASS_TRACE",
                                                             "0"))))
    LAST_EXEC_NS = res.exec_time_ns
    outs = [res.results[i]["out"].astype(np.float32) for i in range(NCORES)]
    full = np.concatenate(outs, axis=0).reshape(B, S, HID)
    return full


# revision 46
# speedup vs baseline: 1.4409x; 1.4409x over previous
"""Distributed Trainium2 attention kernel (8 NeuronCores, head-sharded TP).

Reference computation (per problem spec, hardcoded shapes):
  B=2, S=2048, HID=2048, H=32 q-heads, HKV=8 kv-heads, D=64, GQA ratio 4.
  q/k/v projections -> RoPE(q,k) -> causal softmax attention -> out proj wo.

Sharding: tensor-parallel over heads. Core i owns q-heads 4i..4i+3 and
kv-head i (GQA groups stay aligned). Everything on-device is computed in a
transposed layout ([feature, seq]) so no transposes are needed anywhere
except a PE-transpose (identity matmul) for V.

  - host passes hidden in [HID-chunk, group, 128, 1024] contiguous blocks,
    wqT [HID, 256], wkvT [HID, 128], woT [H*D, HID] (all bf16), rope tables
    cosT/sinT in [d, s]-major layout with rotate-half signs folded in, and a
    single [128,128] triangular causal band mask.
  - qT = wqT.T @ hiddenT, kT/vT packed: [wk|wv].T @ hiddenT (PSUM accum)
  - RoPE in [d, s] layout via partition-swap DMA + 3 DVE ops.
  - attention processes HEAD PAIRS: two scoresT matmuls (contraction K=64)
    run concurrently in the PE array at row offsets 0/64, into one
    [128, 1024] PSUM pair; one ACT exp covers both; causal masking is a
    [128,128] band add pre-exp plus bf16 zero-memsets post-exp.
  - PV: stationary [v | ones] [128, 65] -> out2T [65, 512] PSUM accum;
    row 64 is the softmax denominator for free.
  - divide: DVE reciprocal -> DMA to partition 0 -> gpsimd
    partition_broadcast -> DVE mul (bf16)
  - TWO AllToAlls (head pair 0/1) convert head-sharding -> seq-sharding;
    the first overlaps the second half of attention.
  - wo matmul accumulates interleaved chunks (A2A#1 chunks first) so it can
    start before A2A#2 completes. Host concatenates the 8 [512, 2048] f32
    output shards.
"""

import os
import sys

import numpy as np
import ml_dtypes

sys.path.insert(0, "/opt/trn_rl_repo")

import concourse.bass as bass  # noqa: E402
import concourse.mybir as mybir  # noqa: E402
import concourse.tile as tile  # noqa: E402
from concourse import bacc  # noqa: E402
from concourse.bass_utils import run_bass_kernel_spmd  # noqa: E402

F32 = mybir.dt.float32
BF16 = mybir.dt.bfloat16

H, HKV, D = 32, 8, 64
HID = 2048
B, S = 2, 2048
BS = B * S            # 4096 flattened (b, s)
NCORES = 8
NHQ = H // NCORES     # 4 local q heads
MQ = NHQ * D          # 256 local q rows
SCHUNK = BS // NCORES  # 512 output rows per core
NEG = -30000.0

LAST_EXEC_NS = None


def _build():
    nc = bacc.Bacc("TRN2", target_bir_lowering=False, debug=False,
                   num_devices=NCORES)

    hidden4 = nc.dram_tensor("hidden4", [8, 16, 128, 512], BF16,
                             kind="ExternalInput")
    wqT = nc.dram_tensor("wqT", [HID, MQ], BF16, kind="ExternalInput")
    wkvT = nc.dram_tensor("wkvT", [HID, 2 * D], BF16, kind="ExternalInput")
    woT = nc.dram_tensor("woT", [H * D, HID], BF16, kind="ExternalInput")
    cosT = nc.dram_tensor("cosT", [128, BS], BF16, kind="ExternalInput")
    sinT = nc.dram_tensor("sinT", [128, BS], BF16, kind="ExternalInput")
    maskb = nc.dram_tensor("maskb", [128, 128], BF16, kind="ExternalInput")
    ident = nc.dram_tensor("ident", [128, D], BF16, kind="ExternalInput")
    out = nc.dram_tensor("out", [SCHUNK, HID], F32, kind="ExternalOutput")

    with tile.TileContext(nc) as tc:
        with (
            tc.tile_pool(name="persist", bufs=1) as persist,
            tc.tile_pool(name="qkv", bufs=1) as qkv,
            tc.tile_pool(name="wop", bufs=1) as wop,
            tc.tile_pool(name="dram", bufs=1, space="DRAM") as dram,
        ):
            # ---- persistent SBUF loads -------------------------------------
            # matmul weights first on the sync queue (they gate the first
            # projection); rope tables etc. on the scalar queue.
            wq_sb = persist.tile([128, 16, MQ], BF16)
            nc.sync.dma_start(wq_sb, wqT.rearrange("(c p) m -> p c m", p=128))
            wkv_sb = persist.tile([128, 16, 2 * D], BF16)
            nc.scalar.dma_start(wkv_sb,
                                wkvT.rearrange("(c p) m -> p c m", p=128))
            cos_sb = persist.tile([128, BS], BF16)
            cos_dma = nc.sync.dma_start(cos_sb, cosT[:, :])
            sin_sb = persist.tile([128, BS], BF16)
            sin_dma = nc.sync.dma_start(sin_sb, sinT[:, :])
            mask_sb = persist.tile([128, 128], BF16)
            nc.sync.dma_start(mask_sb, maskb[:, :])
            id_sb = persist.tile([128, D], BF16)
            nc.sync.dma_start(id_sb, ident[:, :])

            # wo weights go through the gpsimd SWDGE queue in chunks so their
            # long transfer never sits on a HWDGE queue whose cumulative
            # semaphore gates small latency-critical DMAs.
            wo_sb = wop.tile([128, 16, HID], BF16)

            # ---- qkv outputs ----------------------------------------------
            qrope = qkv.tile([128, 2, BS], BF16)     # [64*(h%2), h//2, b*S+s]
            krope = qkv.tile([128, BS], BF16)        # duplicated both halves
            vT_sb = qkv.tile([128, BS], BF16)        # rows 64:128 used
            # per k-tile block: col 0 = ones (denominator row lands on PSUM
            # partition 0 so partition_broadcast works directly), cols 1:65 = v
            v_sb = qkv.tile([128, B, 16 * (D + 1)], BF16)
            for b in range(B):
                for kt in range(16):
                    nc.vector.memset(v_sb[:, b, kt * 65 + 64: kt * 65 + 65],
                                     1.0)

            cc_in_a = dram.tile([NCORES, 128, 512], BF16)  # heads 0,1
            cc_in_b = dram.tile([NCORES, 128, 512], BF16)  # heads 2,3
            cc_out_a = dram.tile([NCORES * 128, 512], BF16)
            cc_out_b = dram.tile([NCORES * 128, 512], BF16)

            # ================= phase 1: projections + rope ==================
            with (
                tc.tile_pool(name="hstream", bufs=2) as hstream,
                tc.tile_pool(name="ropetmp", bufs=2) as ropetmp,
                tc.tile_pool(name="psA", bufs=2, space="PSUM") as psA,
                tc.tile_pool(name="psT", bufs=2, space="PSUM") as psT,
            ):
                h_dmas = []
                for g in range(8):  # 512-col groups of (b, s)
                    cols = bass.ds(g * 512, 512)
                    ps_q0 = psA.tile([128, 512], F32, tag="q0")
                    ps_q1 = psA.tile([128, 512], F32, tag="q1")
                    ps_kv = psA.tile([128, 512], F32, tag="kv")
                    h_sb = hstream.tile([128, 16, 512], BF16, tag="h")
                    if g == 0:
                        # split the first load so c=0..7 matmuls start sooner
                        nc.scalar.dma_start(
                            h_sb[:, 0:8, :], hidden4[g, 0:8, :, :]
                            .rearrange("c p n -> p c n"))
                        h_dma = nc.scalar.dma_start(
                            h_sb[:, 8:16, :], hidden4[g, 8:16, :, :]
                            .rearrange("c p n -> p c n"))
                    else:
                        h_dma = nc.scalar.dma_start(
                            h_sb, hidden4[g, :, :, :]
                            .rearrange("c p n -> p c n"))
                    h_dmas.append(h_dma)
                    for c in range(16):
                        nc.tensor.matmul(ps_q0, wq_sb[:, c, 0:128],
                                         h_sb[:, c, :],
                                         start=(c == 0), stop=(c == 15))
                        nc.tensor.matmul(ps_q1, wq_sb[:, c, 128:256],
                                         h_sb[:, c, :],
                                         start=(c == 0), stop=(c == 15))
                        nc.tensor.matmul(ps_kv, wkv_sb[:, c, :], h_sb[:, c, :],
                                         start=(c == 0), stop=(c == 15))
                    # rope for q0/q1/kv batched: x = [q0 | q1 | kv] bf16,
                    # one stepped-AP DMA pair does all partition swaps.
                    x = ropetmp.tile([128, 3, 512], BF16, tag="x")
                    nc.scalar.copy(x[:, 0, :], ps_q0)
                    nc.scalar.copy(x[:, 1, :], ps_q1)
                    nc.scalar.copy(x[0:64, 2, :], ps_kv[0:64, :])
                    swap = ropetmp.tile([128, 3, 512], BF16, tag="swap")
                    for blk in range(2):
                        p0 = blk * 64
                        nc.scalar.dma_start(swap[p0: p0 + 32, :, :],
                                            x[p0 + 32: p0 + 64, :, :])
                        nc.scalar.dma_start(swap[p0 + 32: p0 + 64, :, :],
                                            x[p0: p0 + 32, :, :])
                    tmp = ropetmp.tile([128, 3, 512], BF16, tag="tmp")
                    cosp = ropetmp.tile([128, 3, 512], BF16, tag="cosp")
                    for t, rows in ((0, 128), (1, 128), (2, 64)):
                        nc.vector.tensor_mul(tmp[:rows, t, :],
                                             swap[:rows, t, :],
                                             sin_sb[:rows, cols])
                        nc.vector.tensor_mul(cosp[:rows, t, :],
                                             x[:rows, t, :],
                                             cos_sb[:rows, cols])
                    nc.vector.tensor_add(qrope[:, 0, cols], tmp[:, 0, :],
                                         cosp[:, 0, :])
                    nc.vector.tensor_add(qrope[:, 1, cols], tmp[:, 1, :],
                                         cosp[:, 1, :])
                    nc.vector.tensor_add(krope[0:64, cols], tmp[0:64, 2, :],
                                         cosp[0:64, 2, :])
                    nc.scalar.copy(vT_sb[64:128, cols], ps_kv[64:128, :])
                    b = g // 4
                    for j in range(4):
                        st = g * 512 + j * 128
                        kt = (st - b * S) // 128
                        tp = psT.tile([128, D], BF16, tag="tp")
                        nc.tensor.transpose(tp,
                                            vT_sb[64:128, bass.ds(st, 128)],
                                            id_sb[64:128, :])
                        nc.vector.tensor_copy(
                            v_sb[:, b, kt * 65: kt * 65 + 64], tp)
                # rope tables aren't needed until the first rope (~20us
                # in); keep their transfers out of the startup critical path
                from concourse.tile import add_dep_helper as _adh
                _adh(cos_dma.ins, h_dmas[0].ins, sync=True,
                     reason="cos table after first hidden group")
                _adh(sin_dma.ins, h_dmas[1].ins, sync=True,
                     reason="sin table after second hidden group")
                # duplicate k into partitions 64:128 (one DMA per batch)
                for b in range(B):
                    bc = bass.ds(b * S, S)
                    nc.sync.dma_start(krope[64:128, bc], krope[0:64, bc])

            # woT load (SWDGE, chunked); held behind the last hidden load
            # so its transfers don't starve the phase-1 stream.
            from concourse.tile import add_dep_helper
            for ch in range(4):
                wo_dma = nc.gpsimd.dma_start(
                    wo_sb[:, bass.ts(ch, 4), :],
                    woT.rearrange("(c p) n -> p c n", p=128)[:,
                                                            bass.ts(ch, 4), :])
                add_dep_helper(wo_dma.ins, h_dmas[4 + ch].ins, sync=True,
                               reason="woT chunk interleaves hidden tail")



            # ================= phase 2: attention ===========================
            # head pairs: hp=0 -> heads 0,1 (qrope m=0), hp=1 -> heads 2,3
            with (
                tc.tile_pool(name="attnp", bufs=4) as attnp,
                tc.tile_pool(name="divp", bufs=3) as divp,
                tc.tile_pool(name="psS", bufs=3, space="PSUM") as psS,
                tc.tile_pool(name="psO", bufs=1, space="PSUM") as psO,
            ):
                for hp in range(2):
                    cc_tile = cc_in_a if hp == 0 else cc_in_b
                    for b in range(B):
                        for qc in range(4):
                            qcols = bass.ds(b * S + qc * 512, 512)
                            nkt = 4 * qc + 4
                            ps_oa = psO.tile([65, 512], F32, tag="poa")
                            ps_ob = psO.tile([65, 512], F32, tag="pob")
                            for kt in range(nkt):
                                kcols = bass.ds(b * S + kt * 128, 128)
                                ps_s = psS.tile([128, 1024], F32, tag="pss")
                                nc.tensor.matmul(ps_s[:, 0:512],
                                                 krope[0:64, kcols],
                                                 qrope[0:64, hp, qcols],
                                                 start=True, stop=True)
                                nc.tensor.matmul(ps_s[:, 512:1024],
                                                 krope[64:128, kcols],
                                                 qrope[64:128, hp, qcols],
                                                 start=True, stop=True)
                                j = kt - 4 * qc
                                attn = attnp.tile([128, 1024], BF16,
                                                  tag="attn")
                                nc.scalar.activation(
                                    attn, ps_s,
                                    mybir.ActivationFunctionType.Exp,
                                    scale=0.125)
                                if j >= 0:
                                    # causal band: multiply by 0/1 mask
                                    # post-exp (keeps DVE off the
                                    # scores->exp critical path)
                                    for half in range(2):
                                        o = half * 512 + j * 128
                                        nc.vector.tensor_mul(
                                            attn[:, o:o + 128],
                                            attn[:, o:o + 128], mask_sb)
                                if j >= 1:
                                    for half in range(2):
                                        o = half * 512
                                        nc.vector.memset(
                                            attn[:, o:o + j * 128], 0.0)
                                vs = v_sb[:, b, kt * 65: kt * 65 + 65]
                                nc.tensor.matmul(ps_oa, vs, attn[:, 0:512],
                                                 start=(kt == 0),
                                                 stop=(kt == nkt - 1),
                                                 skip_group_check=True)
                                nc.tensor.matmul(ps_ob, vs, attn[:, 512:1024],
                                                 start=(kt == 0),
                                                 stop=(kt == nkt - 1),
                                                 skip_group_check=True)
                            for half, ps_o in ((0, ps_oa), (1, ps_ob)):
                                # copy out of PSUM right away so the bank is
                                # free for the next q-chunk's accumulation
                                osb = divp.tile([65, 512], F32, tag="osb")
                                nc.vector.tensor_copy(osb, ps_o)
                                ps_o = osb
                                recip = divp.tile([65, 512], F32, tag="recip")
                                nc.vector.reciprocal(recip[64:65, :],
                                                     ps_o[64:65, :])
                                # broadcast across partitions via a stride-0
                                # DRAM read (keeps the Pool engine free for
                                # the collectives)
                                den_d = dram.tile([1, 512], F32, tag="den",
                                                  bufs=4, name="den_d")
                                nc.scalar.dma_start(den_d, recip[64:65, :])
                                rbc = divp.tile([64, 512], F32, tag="rbc")
                                nc.scalar.dma_start(
                                    rbc, den_d[0:1, :].broadcast_to((64, 512)))
                                ao = divp.tile([64, 512], BF16, tag="ao")
                                nc.vector.tensor_mul(ao, ps_o[0:64, :], rbc)
                                nc.sync.dma_start(
                                    cc_tile[b * 4 + qc,
                                            bass.ts(half, 64), :], ao)
                    # fire the A2A for this head pair as soon as its last
                    # bounce write lands; hp=0's overlaps hp=1's attention.
                    nc.gpsimd.collective_compute(
                        "AllToAll", mybir.AluOpType.bypass,
                        replica_groups=[list(range(NCORES))],
                        ins=[(cc_in_a if hp == 0 else cc_in_b).opt()],
                        outs=[(cc_out_a if hp == 0 else cc_out_b).opt()])
                    # aot loads via SWDGE (off the HWDGE queues so their
                    # A2A gating can't poison cumulative queue semaphores);
                    # emitted here so the trigger isn't stuck behind the
                    # next A2A's input wait on the Pool SEQ.
                    if hp == 0:
                        aot_a = wop.tile([128, 8, 512], BF16)
                        nc.gpsimd.dma_start(
                            aot_a,
                            cc_out_a.rearrange("(c p) n -> p c n", p=128))
                    else:
                        aot_b = wop.tile([128, 8, 512], BF16)
                        nc.gpsimd.dma_start(
                            aot_b,
                            cc_out_b.rearrange("(c p) n -> p c n", p=128))


            # ================= phase 3: wo ==================================
            with (
                tc.tile_pool(name="psW", bufs=2, space="PSUM") as psW,
                tc.tile_pool(name="outp", bufs=2) as outp,
            ):
                # two st-waves of 2; within a wave all A2A#1 (even) chunks
                # first so the PE isn't head-of-line blocked on A2A#2.
                for wave in range(2):
                    pws = [psW.tile([128, HID], F32, tag=f"psw{w}", bufs=1,
                                    name=f"psw_{wave}_{w}")
                           for w in range(2)]
                    for phase, aot in ((0, aot_a), (1, aot_b)):
                        for w in range(2):
                            st = wave * 2 + w
                            for i in range(8):
                                c = 2 * i + phase
                                for nh in range(4):
                                    ns = bass.ts(nh, 512)
                                    nc.tensor.matmul(
                                        pws[w][:, ns],
                                        aot[:, i, bass.ts(st, 128)],
                                        wo_sb[:, c, ns],
                                        start=(phase == 0 and i == 0),
                                        stop=(phase == 1 and i == 7))
                    for w in range(2):
                        st = wave * 2 + w
                        osb = outp.tile([128, HID], F32, tag="osb")
                        nc.vector.tensor_copy(osb, pws[w])
                        nc.sync.dma_start(out[bass.ts(st, 128), :], osb)

    nc.compile()
    return nc


_NC_CACHE = {}


def _get_nc():
    if "nc" not in _NC_CACHE:
        _NC_CACHE["nc"] = _build()
    return _NC_CACHE["nc"]


def _prep_inputs(hidden_states, cos, sin, wq, wk, wv, wo):
    bf = ml_dtypes.bfloat16
    hiddenT = np.ascontiguousarray(
        hidden_states.reshape(BS, HID).T).astype(bf)       # [HID, BS]
    hidden4 = np.ascontiguousarray(
        hiddenT.reshape(16, 128, 8, 512).transpose(2, 0, 1, 3))
    woT = np.ascontiguousarray(np.asarray(wo).T).astype(bf)

    cos2 = np.asarray(cos)[:, 0, :]          # [S, D]
    sin2 = np.asarray(sin)[:, 0, :]
    cosTb = cos2.T                            # [D, S]
    sinTb = sin2.T
    sin_signed = np.concatenate([-sinTb[:32], sinTb[32:]], axis=0)
    cos_full = np.tile(cosTb, (2, B)).astype(bf)       # [128, B*S]
    sin_full = np.tile(sin_signed, (2, B)).astype(bf)  # [128, B*S]

    # triangular causal band mask for the diagonal [128k x 128q] block
    kk = np.arange(128)[:, None]
    qq = np.arange(128)[None, :]
    maskb = np.where(kk > qq, 0.0, 1.0).astype(np.float32).astype(bf)

    ident_np = np.zeros((128, D), np.float32)
    ident_np[64:128, :] = np.eye(D)
    ident_np = ident_np.astype(bf)

    wq = np.asarray(wq)
    wk = np.asarray(wk)
    wv = np.asarray(wv)
    in_maps = []
    for i in range(NCORES):
        wq_i = wq[i * MQ:(i + 1) * MQ, :]                      # [256, HID]
        wkv_i = np.concatenate([wk[i * D:(i + 1) * D, :],
                                wv[i * D:(i + 1) * D, :]], axis=0)
        in_maps.append({
            "hidden4": hidden4,
            "wqT": np.ascontiguousarray(wq_i.T).astype(bf),
            "wkvT": np.ascontiguousarray(wkv_i.T).astype(bf),
            "woT": woT,
            "cosT": cos_full,
            "sinT": sin_full,
            "maskb": maskb,
            "ident": ident_np,
        })
    return in_maps


def kernel(hidden_states, cos, sin, wq, wk, wv, wo):
    global LAST_EXEC_NS
    in_maps = _prep_inputs(np.asarray(hidden_states, np.float32),
                           cos, sin, wq, wk, wv, wo)
    nc = _get_nc()
    res = run_bass_kernel_spmd(nc, in_maps, core_ids=list(range(NCORES)),
                               trace=bool(int(os.environ.get("BASS_TRACE",
                                                             "0"))))
    LAST_EXEC_NS = res.exec_time_ns
    outs = [res.results[i]["out"].astype(np.float32) for i in range(NCORES)]
    full = np.concatenate(outs, axis=0).reshape(B, S, HID)
    return full


# revision 49
# speedup vs baseline: 2.8857x; 2.0027x over previous
"""Distributed Trainium2 attention kernel (8 NeuronCores, head-sharded TP).

Reference computation (per problem spec, hardcoded shapes):
  B=2, S=2048, HID=2048, H=32 q-heads, HKV=8 kv-heads, D=64, GQA ratio 4.
  q/k/v projections -> RoPE(q,k) -> causal softmax attention -> out proj wo.

Sharding: tensor-parallel over heads. Core i owns q-heads 4i..4i+3 and
kv-head i (GQA groups stay aligned). Everything on-device is computed in a
transposed layout ([feature, seq]) so no transposes are needed anywhere
except a PE-transpose (identity matmul) for V.

  - host passes hidden as [group, HID-chunk, 128, 512] contiguous blocks,
    wqT [HID, 256], wkvT [HID, 128], woT [H*D, HID] (all bf16), rope tables
    cosT/sinT in [d, s]-major layout with rotate-half signs folded in, and a
    [128,128] 0/1 triangular causal band mask.
  - qT = wqT.T @ hiddenT, kT/vT packed: [wk|wv].T @ hiddenT (PSUM accum,
    one 2.1MB DMA per group; few/large DMAs because every DMA instruction
    costs ~0.6us of shared HWDGE descriptor-gen time)
  - RoPE in [d, s] layout: ACT copy to bf16, partition-swap DMA, 3 DVE ops.
  - V is PE-transposed (identity matmul at base partition 64) into [s, d]
    blocks of [ v | ones-column ].
  - attention processes HEAD PAIRS: two scoresT matmuls (contraction K=64)
    run concurrently in the PE array at row offsets 0/64, into one
    [128, 1024] PSUM pair; one ACT exp (scale=1/8 folded, no
    max-subtraction since logits are ~N(0,1)) covers both heads; causal
    masking is a post-exp 0/1 band multiply + zero-memsets, keeping the
    DVE off the scores->exp critical path.
  - PV: stationary [v | ones] [128, 65] -> out2T [65, 512] PSUM accum;
    row 64 is the softmax denominator for free. The accumulator is copied
    to SBUF immediately so the PSUM bank frees for the next q-chunk.
  - divide: DVE reciprocal -> DRAM bounce -> stride-0 broadcast DMA ->
    DVE mul (bf16). (gpsimd.partition_broadcast is avoided: collectives
    retire late on the Pool engine and poison its cumulative instruction
    semaphore for every later Pool-sem waiter.)
  - TWO AllToAlls (head pairs) convert head-sharding -> seq-sharding; the
    first overlaps the second half of attention. aot/woT loads ride the
    gpsimd SWDGE queue so their collective-gated completion can't poison
    HWDGE cumulative queue semaphores that attention-critical DMAs wait on.
  - wo matmul: per 128-row output tile, A2A#1 chunks (global-even) for ALL
    st first, then A2A#2 chunks, so the PE is never head-of-line blocked
    on the second collective. Host concatenates the 8 [512, 2048] f32
    output shards.

Timing feedback: NTFF/neuron-profile is unavailable under this axon client,
so optimization was driven by the Tile cost model (TimelineSim) with true
per-instruction span extraction; end state ~459us predicted, dominated by
PE busy (~319us) + the modeled A2A#2 tail.
"""

import os
import sys

import numpy as np
import ml_dtypes

sys.path.insert(0, "/opt/trn_rl_repo")

import concourse.bass as bass  # noqa: E402
import concourse.mybir as mybir  # noqa: E402
import concourse.tile as tile  # noqa: E402
from concourse import bacc  # noqa: E402
from concourse.bass_utils import run_bass_kernel_spmd  # noqa: E402

F32 = mybir.dt.float32
BF16 = mybir.dt.bfloat16

H, HKV, D = 32, 8, 64
HID = 2048
B, S = 2, 2048
BS = B * S            # 4096 flattened (b, s)
NCORES = 8
NHQ = H // NCORES     # 4 local q heads
MQ = NHQ * D          # 256 local q rows
SCHUNK = BS // NCORES  # 512 output rows per core
NEG = -30000.0

LAST_EXEC_NS = None


def _build():
    nc = bacc.Bacc("TRN2", target_bir_lowering=False, debug=False,
                   num_devices=NCORES)

    hidden4 = nc.dram_tensor("hidden4", [8, 16, 128, 512], BF16,
                             kind="ExternalInput")
    wqT = nc.dram_tensor("wqT", [HID, MQ], BF16, kind="ExternalInput")
    wkvT = nc.dram_tensor("wkvT", [HID, 2 * D], BF16, kind="ExternalInput")
    woT = nc.dram_tensor("woT", [H * D, HID], BF16, kind="ExternalInput")
    cosT = nc.dram_tensor("cosT", [128, BS], BF16, kind="ExternalInput")
    sinT = nc.dram_tensor("sinT", [128, BS], BF16, kind="ExternalInput")
    maskb = nc.dram_tensor("maskb", [128, 128], BF16, kind="ExternalInput")
    ident = nc.dram_tensor("ident", [128, D], BF16, kind="ExternalInput")
    out = nc.dram_tensor("out", [SCHUNK, HID], F32, kind="ExternalOutput")

    with tile.TileContext(nc) as tc:
        with (
            tc.tile_pool(name="persist", bufs=1) as persist,
            tc.tile_pool(name="qkv", bufs=1) as qkv,
            tc.tile_pool(name="wop", bufs=1) as wop,
            tc.tile_pool(name="dram", bufs=1, space="DRAM") as dram,
        ):
            # ---- persistent SBUF loads -------------------------------------
            # matmul weights first on the sync queue (they gate the first
            # projection); rope tables etc. on the scalar queue.
            wq_sb = persist.tile([128, 16, MQ], BF16)
            nc.sync.dma_start(wq_sb, wqT.rearrange("(c p) m -> p c m", p=128))
            wkv_sb = persist.tile([128, 16, 2 * D], BF16)
            nc.scalar.dma_start(wkv_sb,
                                wkvT.rearrange("(c p) m -> p c m", p=128))
            cos_sb = persist.tile([128, BS], BF16)
            cos_dma = nc.sync.dma_start(cos_sb, cosT[:, :])
            sin_sb = persist.tile([128, BS], BF16)
            sin_dma = nc.sync.dma_start(sin_sb, sinT[:, :])
            mask_sb = persist.tile([128, 128], BF16)
            nc.sync.dma_start(mask_sb, maskb[:, :])
            id_sb = persist.tile([128, D], BF16)
            nc.sync.dma_start(id_sb, ident[:, :])

            # wo weights go through the gpsimd SWDGE queue in chunks so their
            # long transfer never sits on a HWDGE queue whose cumulative
            # semaphore gates small latency-critical DMAs.
            wo_sb = wop.tile([128, 16, HID], BF16)

            # ---- qkv outputs ----------------------------------------------
            qrope = qkv.tile([128, 2, BS], BF16)     # [64*(h%2), h//2, b*S+s]
            krope = qkv.tile([128, BS], BF16)        # duplicated both halves
            vT_sb = qkv.tile([128, BS], BF16)        # rows 64:128 used
            # per k-tile block: col 0 = ones (denominator row lands on PSUM
            # partition 0 so partition_broadcast works directly), cols 1:65 = v
            v_sb = qkv.tile([128, B, 16 * (D + 1)], BF16)
            for b in range(B):
                for kt in range(16):
                    nc.vector.memset(v_sb[:, b, kt * 65 + 64: kt * 65 + 65],
                                     1.0)

            cc_in_a = dram.tile([NCORES, 128, 512], BF16)  # heads 0,1
            cc_in_b = dram.tile([NCORES, 128, 512], BF16)  # heads 2,3
            cc_out_a = dram.tile([NCORES * 128, 512], BF16)
            cc_out_b = dram.tile([NCORES * 128, 512], BF16)

            # ================= phase 1: projections + rope ==================
            with (
                tc.tile_pool(name="hstream", bufs=2) as hstream,
                tc.tile_pool(name="ropetmp", bufs=2) as ropetmp,
                tc.tile_pool(name="psA", bufs=2, space="PSUM") as psA,
                tc.tile_pool(name="psT", bufs=2, space="PSUM") as psT,
            ):
                h_dmas = []
                for g in range(8):  # 512-col groups of (b, s)
                    cols = bass.ds(g * 512, 512)
                    ps_q0 = psA.tile([128, 512], F32, tag="q0")
                    ps_q1 = psA.tile([128, 512], F32, tag="q1")
                    ps_kv = psA.tile([128, 512], F32, tag="kv")
                    h_sb = hstream.tile([128, 16, 512], BF16, tag="h")
                    if g == 0:
                        # split the first load so c=0..7 matmuls start sooner
                        nc.scalar.dma_start(
                            h_sb[:, 0:8, :], hidden4[g, 0:8, :, :]
                            .rearrange("c p n -> p c n"))
                        h_dma = nc.scalar.dma_start(
                            h_sb[:, 8:16, :], hidden4[g, 8:16, :, :]
                            .rearrange("c p n -> p c n"))
                    else:
                        h_dma = nc.scalar.dma_start(
                            h_sb, hidden4[g, :, :, :]
                            .rearrange("c p n -> p c n"))
                    h_dmas.append(h_dma)
                    for c in range(16):
                        nc.tensor.matmul(ps_q0, wq_sb[:, c, 0:128],
                                         h_sb[:, c, :],
                                         start=(c == 0), stop=(c == 15))
                        nc.tensor.matmul(ps_q1, wq_sb[:, c, 128:256],
                                         h_sb[:, c, :],
                                         start=(c == 0), stop=(c == 15))
                        nc.tensor.matmul(ps_kv, wkv_sb[:, c, :], h_sb[:, c, :],
                                         start=(c == 0), stop=(c == 15))
                    # rope for q0/q1/kv batched: x = [q0 | q1 | kv] bf16,
                    # one stepped-AP DMA pair does all partition swaps.
                    x = ropetmp.tile([128, 3, 512], BF16, tag="x")
                    nc.scalar.copy(x[:, 0, :], ps_q0)
                    nc.scalar.copy(x[:, 1, :], ps_q1)
                    nc.scalar.copy(x[0:64, 2, :], ps_kv[0:64, :])
                    swap = ropetmp.tile([128, 3, 512], BF16, tag="swap")
                    for blk in range(2):
                        p0 = blk * 64
                        nc.scalar.dma_start(swap[p0: p0 + 32, :, :],
                                            x[p0 + 32: p0 + 64, :, :])
                        nc.scalar.dma_start(swap[p0 + 32: p0 + 64, :, :],
                                            x[p0: p0 + 32, :, :])
                    tmp = ropetmp.tile([128, 3, 512], BF16, tag="tmp")
                    cosp = ropetmp.tile([128, 3, 512], BF16, tag="cosp")
                    for t, rows in ((0, 128), (1, 128), (2, 64)):
                        nc.vector.tensor_mul(tmp[:rows, t, :],
                                             swap[:rows, t, :],
                                             sin_sb[:rows, cols])
                        nc.vector.tensor_mul(cosp[:rows, t, :],
                                             x[:rows, t, :],
                                             cos_sb[:rows, cols])
                    nc.vector.tensor_add(qrope[:, 0, cols], tmp[:, 0, :],
                                         cosp[:, 0, :])
                    nc.vector.tensor_add(qrope[:, 1, cols], tmp[:, 1, :],
                                         cosp[:, 1, :])
                    nc.vector.tensor_add(krope[0:64, cols], tmp[0:64, 2, :],
                                         cosp[0:64, 2, :])
                    nc.scalar.copy(vT_sb[64:128, cols], ps_kv[64:128, :])
                    b = g // 4
                    for j in range(4):
                        st = g * 512 + j * 128
                        kt = (st - b * S) // 128
                        tp = psT.tile([128, D], BF16, tag="tp")
                        nc.tensor.transpose(tp,
                                            vT_sb[64:128, bass.ds(st, 128)],
                                            id_sb[64:128, :])
                        nc.vector.tensor_copy(
                            v_sb[:, b, kt * 65: kt * 65 + 64], tp)
                # rope tables aren't needed until the first rope (~20us
                # in); keep their transfers out of the startup critical path
                from concourse.tile import add_dep_helper as _adh
                _adh(cos_dma.ins, h_dmas[0].ins, sync=True,
                     reason="cos table after first hidden group")
                _adh(sin_dma.ins, h_dmas[1].ins, sync=True,
                     reason="sin table after second hidden group")
                # duplicate k into partitions 64:128 (one DMA per batch)
                for b in range(B):
                    bc = bass.ds(b * S, S)
                    nc.sync.dma_start(krope[64:128, bc], krope[0:64, bc])

            # woT load (SWDGE, chunked); held behind the last hidden load
            # so its transfers don't starve the phase-1 stream.
            from concourse.tile import add_dep_helper
            for ch in range(4):
                wo_dma = nc.gpsimd.dma_start(
                    wo_sb[:, bass.ts(ch, 4), :],
                    woT.rearrange("(c p) n -> p c n", p=128)[:,
                                                            bass.ts(ch, 4), :])
                add_dep_helper(wo_dma.ins, h_dmas[4 + ch].ins, sync=True,
                               reason="woT chunk interleaves hidden tail")



            # ================= phase 2: attention ===========================
            # head pairs: hp=0 -> heads 0,1 (qrope m=0), hp=1 -> heads 2,3
            with (
                tc.tile_pool(name="attnp", bufs=4) as attnp,
                tc.tile_pool(name="divp", bufs=3) as divp,
                tc.tile_pool(name="psS", bufs=3, space="PSUM") as psS,
                tc.tile_pool(name="psO", bufs=1, space="PSUM") as psO,
            ):
                for hp in range(2):
                    cc_tile = cc_in_a if hp == 0 else cc_in_b
                    for b in range(B):
                        for qc in range(4):
                            qcols = bass.ds(b * S + qc * 512, 512)
                            nkt = 4 * qc + 4
                            ps_oa = psO.tile([65, 512], F32, tag="poa")
                            ps_ob = psO.tile([65, 512], F32, tag="pob")
                            for kt in range(nkt):
                                kcols = bass.ds(b * S + kt * 128, 128)
                                ps_s = psS.tile([128, 1024], F32, tag="pss")
                                nc.tensor.matmul(ps_s[:, 0:512],
                                                 krope[0:64, kcols],
                                                 qrope[0:64, hp, qcols],
                                                 start=True, stop=True)
                                nc.tensor.matmul(ps_s[:, 512:1024],
                                                 krope[64:128, kcols],
                                                 qrope[64:128, hp, qcols],
                                                 start=True, stop=True)
                                j = kt - 4 * qc
                                attn = attnp.tile([128, 1024], BF16,
                                                  tag="attn")
                                nc.scalar.activation(
                                    attn, ps_s,
                                    mybir.ActivationFunctionType.Exp,
                                    scale=0.125)
                                if j >= 0:
                                    # causal band: multiply by 0/1 mask
                                    # post-exp (keeps DVE off the
                                    # scores->exp critical path)
                                    for half in range(2):
                                        o = half * 512 + j * 128
                                        nc.vector.tensor_mul(
                                            attn[:, o:o + 128],
                                            attn[:, o:o + 128], mask_sb)
                                if j >= 1:
                                    for half in range(2):
                                        o = half * 512
                                        nc.vector.memset(
                                            attn[:, o:o + j * 128], 0.0)
                                vs = v_sb[:, b, kt * 65: kt * 65 + 65]
                                nc.tensor.matmul(ps_oa, vs, attn[:, 0:512],
                                                 start=(kt == 0),
                                                 stop=(kt == nkt - 1),
                                                 skip_group_check=True)
                                nc.tensor.matmul(ps_ob, vs, attn[:, 512:1024],
                                                 start=(kt == 0),
                                                 stop=(kt == nkt - 1),
                                                 skip_group_check=True)
                            for half, ps_o in ((0, ps_oa), (1, ps_ob)):
                                # copy out of PSUM right away so the bank is
                                # free for the next q-chunk's accumulation
                                osb = divp.tile([65, 512], F32, tag="osb")
                                nc.vector.tensor_copy(osb, ps_o)
                                ps_o = osb
                                recip = divp.tile([65, 512], F32, tag="recip")
                                nc.vector.reciprocal(recip[64:65, :],
                                                     ps_o[64:65, :])
                                # broadcast across partitions via a stride-0
                                # DRAM read (keeps the Pool engine free for
                                # the collectives)
                                den_d = dram.tile([1, 512], F32, tag="den",
                                                  bufs=4, name="den_d")
                                nc.scalar.dma_start(den_d, recip[64:65, :])
                                rbc = divp.tile([64, 512], F32, tag="rbc")
                                nc.scalar.dma_start(
                                    rbc, den_d[0:1, :].broadcast_to((64, 512)))
                                ao = divp.tile([64, 512], BF16, tag="ao")
                                nc.vector.tensor_mul(ao, ps_o[0:64, :], rbc)
                                nc.sync.dma_start(
                                    cc_tile[b * 4 + qc,
                                            bass.ts(half, 64), :], ao)
                    # fire the A2A for this head pair as soon as its last
                    # bounce write lands; hp=0's overlaps hp=1's attention.
                    nc.gpsimd.collective_compute(
                        "AllToAll", mybir.AluOpType.bypass,
                        replica_groups=[list(range(NCORES))],
                        ins=[(cc_in_a if hp == 0 else cc_in_b).opt()],
                        outs=[(cc_out_a if hp == 0 else cc_out_b).opt()])
                    # aot loads via SWDGE (off the HWDGE queues so their
                    # A2A gating can't poison cumulative queue semaphores);
                    # emitted here so the trigger isn't stuck behind the
                    # next A2A's input wait on the Pool SEQ.
                    if hp == 0:
                        aot_a = wop.tile([128, 8, 512], BF16)
                        nc.gpsimd.dma_start(
                            aot_a,
                            cc_out_a.rearrange("(c p) n -> p c n", p=128))
                    else:
                        aot_b = wop.tile([128, 8, 512], BF16)
                        nc.gpsimd.dma_start(
                            aot_b,
                            cc_out_b.rearrange("(c p) n -> p c n", p=128))


            # ================= phase 3: wo ==================================
            with (
                tc.tile_pool(name="psW", bufs=2, space="PSUM") as psW,
                tc.tile_pool(name="outp", bufs=2) as outp,
            ):
                # two st-waves of 2; within a wave all A2A#1 (even) chunks
                # first so the PE isn't head-of-line blocked on A2A#2.
                for wave in range(2):
                    pws = [psW.tile([128, HID], F32, tag=f"psw{w}", bufs=1,
                                    name=f"psw_{wave}_{w}")
                           for w in range(2)]
                    for phase, aot in ((0, aot_a), (1, aot_b)):
                        for w in range(2):
                            st = wave * 2 + w
                            for i in range(8):
                                c = 2 * i + phase
                                for nh in range(4):
                                    ns = bass.ts(nh, 512)
                                    nc.tensor.matmul(
                                        pws[w][:, ns],
                                        aot[:, i, bass.ts(st, 128)],
                                        wo_sb[:, c, ns],
                                        start=(phase == 0 and i == 0),
                                        stop=(phase == 1 and i == 7))
                    for w in range(2):
                        st = wave * 2 + w
                        osb = outp.tile([128, HID], F32, tag="osb")
                        nc.vector.tensor_copy(osb, pws[w])
                        nc.sync.dma_start(out[bass.ts(st, 128), :], osb)

    nc.compile()
    return nc


_NC_CACHE = {}


def _get_nc():
    if "nc" not in _NC_CACHE:
        _NC_CACHE["nc"] = _build()
    return _NC_CACHE["nc"]


def _prep_inputs(hidden_states, cos, sin, wq, wk, wv, wo):
    bf = ml_dtypes.bfloat16
    hiddenT = np.ascontiguousarray(
        hidden_states.reshape(BS, HID).T).astype(bf)       # [HID, BS]
    hidden4 = np.ascontiguousarray(
        hiddenT.reshape(16, 128, 8, 512).transpose(2, 0, 1, 3))
    woT = np.ascontiguousarray(np.asarray(wo).T).astype(bf)

    cos2 = np.asarray(cos)[:, 0, :]          # [S, D]
    sin2 = np.asarray(sin)[:, 0, :]
    cosTb = cos2.T                            # [D, S]
    sinTb = sin2.T
    sin_signed = np.concatenate([-sinTb[:32], sinTb[32:]], axis=0)
    cos_full = np.tile(cosTb, (2, B)).astype(bf)       # [128, B*S]
    sin_full = np.tile(sin_signed, (2, B)).astype(bf)  # [128, B*S]

    # triangular causal band mask for the diagonal [128k x 128q] block
    kk = np.arange(128)[:, None]
    qq = np.arange(128)[None, :]
    maskb = np.where(kk > qq, 0.0, 1.0).astype(np.float32).astype(bf)

    ident_np = np.zeros((128, D), np.float32)
    ident_np[64:128, :] = np.eye(D)
    ident_np = ident_np.astype(bf)

    wq = np.asarray(wq)
    wk = np.asarray(wk)
    wv = np.asarray(wv)
    in_maps = []
    for i in range(NCORES):
        wq_i = wq[i * MQ:(i + 1) * MQ, :]                      # [256, HID]
        wkv_i = np.concatenate([wk[i * D:(i + 1) * D, :],
                                wv[i * D:(i + 1) * D, :]], axis=0)
        in_maps.append({
            "hidden4": hidden4,
            "wqT": np.ascontiguousarray(wq_i.T).astype(bf),
            "wkvT": np.ascontiguousarray(wkv_i.T).astype(bf),
            "woT": woT,
            "cosT": cos_full,
            "sinT": sin_full,
            "maskb": maskb,
            "ident": ident_np,
        })
    return in_maps


def kernel(hidden_states, cos, sin, wq, wk, wv, wo):
    global LAST_EXEC_NS
    in_maps = _prep_inputs(np.asarray(hidden_states, np.float32),
                           cos, sin, wq, wk, wv, wo)
    nc = _get_nc()
    res = run_bass_kernel_spmd(nc, in_maps, core_ids=list(range(NCORES)),
                               trace=bool(int(os.environ.get("BASS_TRACE",
                                                             "0"))))
    LAST_EXEC_NS = res.exec_time_ns
    outs = [res.results[i]["out"].astype(np.float32) for i in range(NCORES)]
    full = np.concatenate(outs, axis=0).reshape(B, S, HID)
    return full
